# revision 62
# baseline (speedup 1.0000x reference)
"""NetVLAD forward on 8 Trainium2 NeuronCores.

Reference computation (per batch b):
    logits = conv_w @ x_flat[b]            # [K, N]    (1x1 conv, K=64, C=128, N=4096)
    a      = softmax(logits, axis=K)
    vlad   = a @ x_flat[b].T - sum_n(a) * centroids    # [K, C]
    vlad   = l2norm(vlad, axis=C)          # intra-normalize
    out[b] = l2norm(vlad.reshape(K*C))     # global normalize

Sharding: pure data-parallel over the batch dim (8 batches per core);
conv weight replicated.  No collectives needed.

v2 design (DMA-bound ~24us/core in the timeline model):
  - x is shipped to the device TWICE in fp8-e4m3 (same total bytes as one
    bf16 copy): x1 = [C, N] layout feeding mm1 (logits), and x2 = a
    host-pre-transposed [n%128, n//128, C+1] layout feeding mm2 directly,
    with a -1 column baked in for the -sum(a) term.  This removes the PE
    transpose AND the PSUM->SBUF copies of x^T that dominated v1.
  - mm1 runs mixed-dtype (fp8 x * bf16 w) so the tiny conv weight keeps
    full precision (w quantization error is systematic across n and does
    not average out; x quantization does).
  - softmax over k (free dim): ACT exp (batched 16 chunks); the k-sum as
    a GPSIMD half-add + DVE quarter-add + quarter-sized DVE reduce; the
    1/s scale as a DVE tensor_tensor in a [p, k, chunk] layout whose
    innermost dim is packed bf16 -> qualifies for the 2x_1p DVE perf mode.
  - a (bf16) @ x2 (fp8) accumulates [vlad_raw | -asum] in one PSUM bank
    per batch; tiny epilogue (centroid subtraction + two L2 norms) on the
    host, as in v1.
  - scheduling: PE p-state warmup, x1 loads lead x2 by ~2 batches (tuned
    against the timeline model: every softmax chain completes during the
    DMA stream; late x2s gate only PE mm2 work), compute emitted in
    stream-arrival order so softmax-gated mm2s never head-of-line block
    later batches' logits in the in-order PE queue.
"""

import numpy as np
import ml_dtypes
from contextlib import ExitStack

import concourse.bass as bass
import concourse.bacc as bacc
import concourse.tile as tile
import concourse.mybir as mybir
from concourse import bass_utils

B, C, K = 64, 128, 64
HW = 64 * 64  # N = H*W
NCORES = 8
BPC = B // NCORES  # batches per core
F32 = mybir.dt.float32
BF16 = mybir.dt.bfloat16
FP8 = mybir.dt.float8e4

NCHUNK = 128              # n-columns per chunk (PE partition limit)
CHUNKS = HW // NCHUNK     # 32 chunks per batch
GROUP = 16                # chunks per group (one ACT/DVE batch, 2 psum banks)
NG = CHUNKS // GROUP      # groups per batch = 2


def _netvlad_tile(tc: tile.TileContext, out_d, x1_d, x2_d, w_d):
    nc = tc.nc
    with ExitStack() as ctx:
        const = ctx.enter_context(tc.tile_pool(name="const", bufs=1))
        x1pool = ctx.enter_context(tc.tile_pool(name="x1", bufs=2 * NG * 4))
        x2pool = ctx.enter_context(tc.tile_pool(name="x2", bufs=2 * NG * 4))
        epool = ctx.enter_context(tc.tile_pool(name="e", bufs=3 * NG))
        hpool = ctx.enter_context(tc.tile_pool(name="h", bufs=3 * NG))
        h2pool = ctx.enter_context(tc.tile_pool(name="h2", bufs=3 * NG))
        apool = ctx.enter_context(tc.tile_pool(name="a", bufs=BPC * NG))
        spool = ctx.enter_context(tc.tile_pool(name="s", bufs=6 * NG))
        opool = ctx.enter_context(tc.tile_pool(name="o", bufs=BPC))
        pl_pool = ctx.enter_context(tc.tile_pool(name="pl", bufs=3, space="PSUM"))
        pv_pool = ctx.enter_context(tc.tile_pool(name="pv", bufs=2, space="PSUM"))

        w_sb = const.tile([C, K], BF16)

        outts = []
        x1t = {}  # ib -> [x1 tile per group]
        x2t = {}  # ib -> [x2 tile per group]
        avst = {}  # ib -> [a tile per group]

        def load_x1(ib):
            x1t[ib] = []
            for g in range(NG):
                x1g = x1pool.tile([C, GROUP * NCHUNK], FP8, tag="x1")
                nc.sync.dma_start(
                    out=x1g,
                    in_=x1_d[ib][:, g * GROUP * NCHUNK : (g + 1) * GROUP * NCHUNK],
                )
                x1t[ib].append(x1g)

        def load_x2(ib):
            x2t[ib] = []
            for g in range(NG):
                x2g = x2pool.tile([NCHUNK, GROUP, C + 1], FP8, tag="x2")
                nc.sync.dma_start(
                    out=x2g, in_=x2_d[ib][:, g * GROUP : (g + 1) * GROUP, :]
                )
                x2t[ib].append(x2g)

        def softmax_chain(pl_slice, width, tag, pool_half, pool_h2=False, split_scale=False):
            """exp + normalize `width` chunks of logits; returns the a tile."""
            e = epool.tile([NCHUNK, K, width], BF16, tag=f"e{tag}")
            e_gk = bass.AP(
                tensor=e.tensor, offset=e.offset, ap=[e.ap[0], e.ap[2], e.ap[1]]
            )
            nc.scalar.activation(e_gk, pl_slice, mybir.ActivationFunctionType.Exp)

            # sum over k: GPSIMD (otherwise idle) halves, DVE (2x_1p mode)
            # quarters, then a quarter-sized DVE reduce
            h = hpool.tile([NCHUNK, K // 2, width], BF16, tag=f"h{tag}")
            h2 = h2pool.tile([NCHUNK, K // 4, width], BF16, tag=f"h2{tag}")
            with nc.allow_low_precision(reason="bf16 partial softmax sum; 0.4% on r averages out over n"):
                half_eng = nc.gpsimd if pool_half else nc.vector
                half_eng.tensor_tensor(
                    out=h,
                    in0=e[:, 0 : K // 2, :],
                    in1=e[:, K // 2 : K, :],
                    op=mybir.AluOpType.add,
                )
                (nc.gpsimd if pool_h2 else nc.vector).tensor_tensor(
                    out=h2,
                    in0=h[:, 0 : K // 4, :],
                    in1=h[:, K // 4 : K // 2, :],
                    op=mybir.AluOpType.add,
                )
            a = apool.tile([NCHUNK, K, width], BF16, tag=f"a{tag}")
            nsplit = 2 if split_scale else 1
            hw_ = width // nsplit
            for q in range(nsplit):
                # with nsplit=2 (drain tail): the whole reduce/recip/scale
                # chain runs per half so the first half's mm2s start early
                h2_gk = bass.AP(
                    tensor=h2.tensor,
                    offset=h2.offset + q * hw_,
                    ap=[h2.ap[0], [1, hw_], h2.ap[1]],
                )
                s = spool.tile([NCHUNK, hw_], F32, tag=f"s{tag}q{nsplit}")
                nc.vector.reduce_sum(s, h2_gk, axis=mybir.AxisListType.X)
                r = spool.tile([NCHUNK, hw_], BF16, tag=f"r{tag}q{nsplit}")
                with nc.allow_low_precision(reason="bf16 r enables the 2x DVE mode on the scale; error averages out over n"):
                    nc.vector.reciprocal(r, s)
                r_bh = bass.AP(
                    tensor=r.tensor, offset=r.offset, ap=[r.ap[0], [0, K], [1, hw_]]
                )
                nc.vector.tensor_tensor(
                    out=a[:, :, q * hw_ : (q + 1) * hw_],
                    in0=e[:, :, q * hw_ : (q + 1) * hw_],
                    in1=r_bh,
                    op=mybir.AluOpType.mult,
                )
            return a

        def stage_AB(ib, sub=1):
            # mm1 logits for all groups, then the softmax chains.  sub>1
            # splits each group's softmax into sub slices so the tail chain
            # pipelines at finer granularity (used for the drain batches).
            pls = []
            for g in range(NG):
                pl = pl_pool.tile([NCHUNK, GROUP, K], F32, tag="pl")
                for i in range(GROUP):
                    nc.tensor.matmul(
                        pl[:, i, :],
                        lhsT=x1t[ib][g][:, i * NCHUNK : (i + 1) * NCHUNK],
                        rhs=w_sb,
                        start=True,
                        stop=True,
                    )
                pls.append(pl)

            w_ = GROUP // sub
            avs = []
            for g in range(NG):
                for q in range(sub):
                    avs.append(
                        softmax_chain(
                            pls[g][:, q * w_ : (q + 1) * w_, :],
                            w_,
                            str(w_),
                            pool_half=True,
                            pool_h2=(ib == BPC - 1 and g == NG - 1),
                            split_scale=(ib >= BPC - 2),
                        )
                    )
            avst[ib] = (avs, w_)

        pend_copy = []

        def stage_C(ib, defer_copy=False):
            pv = pv_pool.tile([K, C + 1], F32, tag="pv")  # [vlad_raw | -asum]
            avs, w_ = avst[ib]
            for ch in range(CHUNKS):
                nc.tensor.matmul(
                    pv,
                    lhsT=avs[ch // w_][:, :, ch % w_],
                    rhs=x2t[ib][ch // GROUP][:, ch % GROUP, :],
                    start=(ch == 0),
                    stop=(ch == CHUNKS - 1),
                )
            if defer_copy:
                pend_copy.append(pv)
            else:
                flush_copies()
                outt = opool.tile([K, C + 1], F32)
                nc.scalar.copy(out=outt, in_=pv)
                outts.append(outt)

        def flush_copies():
            while pend_copy:
                pvp = pend_copy.pop(0)
                outt = opool.tile([K, C + 1], F32)
                nc.scalar.copy(out=outt, in_=pvp)
                outts.append(outt)


        # PE p-state warmup: ~3.3us of dummy matmuls on const data ramp the
        # tensor engine to full clock before the first real mm1s issue.
        warm_a = const.tile([NCHUNK, K], BF16)
        warm_b = const.tile([NCHUNK, C + 1], BF16)
        nc.vector.memset(warm_a, 0.0)
        nc.vector.memset(warm_b, 0.0)
        wpv = pv_pool.tile([K, C + 1], F32, tag="pv")
        NWARM = 16
        for i in range(NWARM):
            nc.tensor.matmul(
                wpv, lhsT=warm_a, rhs=warm_b, start=(i == 0), stop=(i == NWARM - 1)
            )
        warm_out = const.tile([K, 1], F32)
        nc.vector.tensor_copy(out=warm_out, in_=wpv[:, 0:1])  # releases wpv

        # DMA stream: x1 loads lead their batch's x2 by ~2 slots so each
        # softmax chain (exp-paced) finishes just as its x2 lands; late x2s
        # then gate only PE mm2 work.  Tuned against the timeline model.
        stream = [
            ("x1", 0), ("w", None), ("x2", 0), ("x1", 1), ("x1", 2), ("x2", 1),
            ("x1", 3), ("x1", 4), ("x2", 2), ("x1", 5), ("x1", 6),
            ("x2", 3), ("x1", 7), ("x2", 4), ("x2", 5), ("x2", 6), ("x2", 7),
        ]
        for kind, ib in stream:
            if kind == "w":
                # GPSIMD SWDGE queue: doesn't displace the x stream on the SP
                # HWDGE queue, lands before mm1[0] needs it
                nc.gpsimd.dma_start(out=w_sb, in_=w_d)
            else:
                (load_x1 if kind == "x1" else load_x2)(ib)

        # compute issue order follows the stream's arrival order: A[b] right
        # after its x1 slot, C[b] one A-slot after its x2 slot (so a
        # softmax-gated C never head-of-line-blocks a ready A in the in-order
        # PE queue).
        for kind, b in [("A", 0), ("A", 1), ("C", 0), ("A", 2), ("C", 1),
                        ("A", 3), ("A", 4), ("C", 2), ("A", 5), ("A", 6),
                        ("C", 3), ("A", 7), ("C", 4), ("C", 5), ("C", 6), ("C", 7)]:
            if kind == "A":
                stage_AB(b)
            else:
                # copies of mid-stream batches would interleave the (saturated)
                # ACT exp queue; defer them until after the last exp issues
                stage_C(b)

        # all output DMAs after the x loads so they never head-of-line block
        # the (bottleneck) input stream on the sync queue
        for ib, outt in enumerate(outts):
            nc.sync.dma_start(out=out_d[ib], in_=outt)


_NC_CACHE = None


def _get_nc():
    global _NC_CACHE
    if _NC_CACHE is None:
        nc = bacc.Bacc(
            "TRN2",
            target_bir_lowering=False,
            debug=False,
            num_devices=NCORES,
        )
        x1_d = nc.dram_tensor("x1", [BPC, C, HW], FP8, kind="ExternalInput").ap()
        x2_d = nc.dram_tensor(
            "x2", [BPC, NCHUNK, CHUNKS, C + 1], FP8, kind="ExternalInput"
        ).ap()
        w_d = nc.dram_tensor("w_t", [C, K], BF16, kind="ExternalInput").ap()
        out_d = nc.dram_tensor("out", [BPC, K, C + 1], F32, kind="ExternalOutput").ap()
        with tile.TileContext(nc) as tc:
            _netvlad_tile(tc, out_d, x1_d, x2_d, w_d)
        nc.compile()
        _NC_CACHE = nc
    return _NC_CACHE


def _make_in_maps(x, conv_w):
    bf16 = ml_dtypes.bfloat16
    f8 = ml_dtypes.float8_e4m3fn
    x1 = np.ascontiguousarray(x.reshape(B, C, HW)).astype(f8)  # [B, C, N]
    # [B, n%128, n//128, C] so mm2's rhs tiles DMA as contiguous rows
    xt = np.ascontiguousarray(
        x1.reshape(B, C, CHUNKS, NCHUNK).transpose(0, 3, 2, 1)
    )
    x2 = np.empty((B, NCHUNK, CHUNKS, C + 1), dtype=f8)
    x2[..., :C] = xt
    x2[..., C] = -1.0
    w_t = np.ascontiguousarray(conv_w.T.astype(bf16))  # [C, K]
    in_maps = []
    for core in range(NCORES):
        sl = slice(core * BPC, (core + 1) * BPC)
        in_maps.append({"x1": x1[sl], "x2": x2[sl], "w_t": w_t})
    return in_maps


def _run(in_maps, trace=False, **kwargs):
    nc = _get_nc()
    return bass_utils.run_bass_kernel_spmd(
        nc, in_maps, core_ids=list(range(NCORES)), trace=trace, **kwargs
    )


def _postprocess(raw, centroids):
    """raw: [B, K, C+1] = [vlad_raw | -asum]  ->  [B, K*C] normalized."""
    vlad = raw[:, :, :C] + raw[:, :, C : C + 1] * centroids[None, :, :]
    norms = np.sqrt((vlad * vlad).sum(axis=2, keepdims=True))
    vlad = vlad / np.maximum(norms, 1e-12)
    out = vlad.reshape(raw.shape[0], K * C)
    gn = np.sqrt((out * out).sum(axis=1, keepdims=True))
    return out / np.maximum(gn, 1e-12)


def kernel(x, conv_w, centroids):
    x = np.asarray(x)
    conv_w = np.asarray(conv_w)
    centroids = np.asarray(centroids, dtype=np.float32)
    res = _run(_make_in_maps(x, conv_w))
    raw = np.concatenate([r["out"] for r in res.results], axis=0)  # [B, K, C+1]
    return _postprocess(raw.astype(np.float32), centroids).astype(np.float32)
